# revision 3
# baseline (speedup 1.0000x reference)
"""CLUB loss kernel for Trainium2 (8 NeuronCores, SPMD row-sharded).

Math: the reference returns mean_i(pos_i - neg_i), a scalar:

  mean_pos = -0.5/N * (A - 2B + C)
      A = sum_{i,d} x^2 * invv,  B = sum x*mu*invv,  C = sum mu^2*invv
  mean_neg = -0.5/N^2 * (S_invv . S_x2 - 2*S_muinvv . S_x + N*C)
  loss = mean_pos - mean_neg

C cancels exactly in the loss, so we never compute it.  The host also
pre-scales mu' = -2*mu, which folds the -2B into a single fused sum:

  P := A - 2B = sum([x | x^2] * [mu'*invv | invv])   (one STT pass)
  loss = -0.5/N * P + 0.5/N^2 * (S_invv.S_x2 + S_mi'.S_x)
      where S_mi' = sum mu'*invv = -2*S_muinvv

Each core handles 2048 rows; layout is d-major (128, 1024): partition
q = (sub-slab b, dim d), free axis = row index, so every reduction is a
free-axis row-sum fused into the producing instruction via accum_out.

Per column-chunk h the SBUF arena tile is laid out
  [ lv | mu' | x | x2 | mi' | invv ]   (6*C cols)
so the DMA lands [lv|mu'|x] as ONE contiguous dma_start (128 descriptors
of 3*C*4 bytes - large packets keep the HWDGE queues at full rate), and
the P pass reads the adjacent [x|x2] and [mi'|invv] spans as single APs.

Compute per chunk (engines balanced; walrus rejects gpsimd STT/TS and
accum, so PL only does its TensorTensor product):
  ACT: invv = exp(-lv) (+Sinvv), x2 = x^2 (+Sx2), Sm copy-acc (chunk 0)
  PL : mi' = mu' * invv
  DVE: P fused STT (+acc), Sx TS (+acc), Sm TS (+acc, chunks 1-2)

Output: acc [128, 15] f32, DMA'd out as two partition-halves on the two
HWDGE queues in parallel; the host combines all cores in float64.
"""

import sys

sys.path.insert(0, "/opt/trn_rl_repo")

import numpy as np
from contextlib import ExitStack

import concourse.bass as bass
import concourse.bacc as bacc
import concourse.tile as tile
from concourse import mybir
from concourse.bass_utils import run_bass_kernel_spmd

F32 = mybir.dt.float32
N_CORES = 8
B, D, H, W = 16, 64, 32, 32
HW = H * W                # 1024
N = B * HW                # 16384
NB = B // N_CORES         # 2 sub-slabs (batches) per core
ROWS = NB * HW            # 2048 rows per core
COLS = HW                 # free size of the (128, 1024) layout
QUANT = ["P", "Sx", "Sm", "Sx2", "Sinvv"]
BOUNDS = [0, 384, 768, 1024]
NCH = len(BOUNDS) - 1
CS = [BOUNDS[h + 1] - BOUNDS[h] for h in range(NCH)]
# which chunks' Sm pass runs on ACT (copy+acc) vs DVE (TS+acc)
SM_ON_ACT = {0}


def build_nc() -> bass.Bass:
    nc = bacc.Bacc()
    ins = [
        nc.dram_tensor(f"in{h}", [128, 3 * CS[h]], F32, kind="ExternalInput")
        for h in range(NCH)
    ]
    accs = nc.dram_tensor("accs", [128, len(QUANT) * NCH], F32,
                          kind="ExternalOutput")

    with ExitStack() as ctx:
        tc = ctx.enter_context(tile.TileContext(nc))
        big = ctx.enter_context(tc.tile_pool(name="big", bufs=1))
        jp = ctx.enter_context(tc.tile_pool(name="jp", bufs=2))
        accp = ctx.enter_context(tc.tile_pool(name="accp", bufs=1))

        zerob = big.tile([128, 1], F32)
        nc.scalar.memzero(zerob[:])

        # per-chunk arena: [lv | mu' | x | x2 | mi' | invv]
        ar = [big.tile([128, 6 * CS[h]], F32, name=f"ar{h}")
              for h in range(NCH)]
        acc = accp.tile([128, len(QUANT) * NCH], F32)

        def col(q, c):
            i = QUANT.index(q) * NCH + c
            return acc[:, i:i + 1]

        qs = [nc.sync, nc.scalar]
        for h in range(NCH):
            C = CS[h]
            qs[h % 2].dma_start(out=ar[h][:, 0:3 * C], in_=ins[h][:, :])

        M = mybir.AluOpType.mult
        ADD = mybir.AluOpType.add
        EXP = mybir.ActivationFunctionType.Exp
        SQ = mybir.ActivationFunctionType.Square
        CP = mybir.ActivationFunctionType.Copy

        for h in range(NCH):
            C = CS[h]
            lv_s = ar[h][:, 0:C]
            mu_s = ar[h][:, C:2 * C]
            x_s = ar[h][:, 2 * C:3 * C]
            x2_s = ar[h][:, 3 * C:4 * C]
            mi_s = ar[h][:, 4 * C:5 * C]
            iv_s = ar[h][:, 5 * C:6 * C]
            xx2_s = ar[h][:, 2 * C:4 * C]   # [x | x2]
            miiv_s = ar[h][:, 4 * C:6 * C]  # [mi' | invv]

            nc.scalar.activation(
                out=iv_s, in_=lv_s, func=EXP, bias=zerob[:], scale=-1.0,
                accum_out=col("Sinvv", h),
            )
            nc.scalar.activation(
                out=x2_s, in_=x_s, func=SQ, bias=zerob[:], scale=1.0,
                accum_out=col("Sx2", h),
            )
            nc.gpsimd.tensor_tensor(mi_s, mu_s, iv_s, op=M)

            # P = sum x*mi' + x2*invv = A - 2B, one fused pass
            jp_t = jp.tile([128, 2 * C], F32, tag="jp", name=f"jp{h}")
            nc.vector.scalar_tensor_tensor(
                out=jp_t[:], in0=xx2_s, scalar=1.0, in1=miiv_s,
                op0=M, op1=M, accum_out=col("P", h),
            )
            jx = jp.tile([128, C], F32, tag="jx", name=f"jx{h}")
            nc.vector.tensor_scalar(
                out=jx[:], in0=x_s, scalar1=1.0, scalar2=0.0,
                op0=M, op1=ADD, accum_out=col("Sx", h),
            )
            if h in SM_ON_ACT:
                jm = jp.tile([128, C], F32, tag="jm", name=f"jm{h}")
                nc.scalar.activation(
                    out=jm[:], in_=mi_s, func=CP, bias=0.0, scale=1.0,
                    accum_out=col("Sm", h),
                )
            else:
                jm = jp.tile([128, C], F32, tag="jm", name=f"jm{h}")
                nc.vector.tensor_scalar(
                    out=jm[:], in0=mi_s, scalar1=1.0, scalar2=0.0,
                    op0=M, op1=ADD, accum_out=col("Sm", h),
                )

        nc.sync.dma_start(out=accs[0:64, :], in_=acc[0:64, :])
        nc.scalar.dma_start(out=accs[64:128, :], in_=acc[64:128, :])
    return nc


def _ensure_ntff_hook():
    """This image's antenv lacks axon_hooks; if tracing is requested
    (e.g. BASS_TRACE=1), run_bass_kernel_spmd would die on the import.
    Register the ctypes-based hook if available, else a None hook so
    tracing degrades gracefully."""
    import types

    if "antenv.axon_hooks" in sys.modules:
        return
    try:
        import antenv.axon_hooks  # noqa: F401
        return
    except ImportError:
        pass
    hook = None
    try:
        sys.path.insert(0, "/root/.axon_site")
        from trn_agent_boot.trn_boot import _ntff_profile_via_ctypes

        hook = _ntff_profile_via_ctypes("/opt/axon/libaxon_pjrt.so")
    except Exception:
        hook = None
    mod = types.ModuleType("antenv.axon_hooks")
    mod._hook = hook
    mod.get_axon_ntff_profile_hook = lambda: mod._hook
    mod.set_axon_ntff_profile_hook = lambda h: setattr(mod, "_hook", h)
    sys.modules["antenv.axon_hooks"] = mod


_ensure_ntff_hook()

_NC = None


def _get_nc():
    global _NC
    if _NC is None:
        _NC = build_nc()
        # bacc passes legalize multi-sync-wait instructions for TRN2 codegen
        _NC.compile()
    return _NC


def make_in_maps(x, mu, logvar):
    x = np.ascontiguousarray(np.asarray(x, dtype=np.float32))
    mu = np.asarray(mu, dtype=np.float32)
    lv = np.asarray(logvar, dtype=np.float32)
    in_maps = []
    for c in range(N_CORES):
        r0 = c * ROWS
        mu_t = np.concatenate(
            [mu[r0 + b * HW:r0 + (b + 1) * HW].T for b in range(NB)], axis=0
        ) * np.float32(-2.0)
        lv_t = np.concatenate(
            [lv[r0 + b * HW:r0 + (b + 1) * HW].T for b in range(NB)], axis=0
        )
        x_t = x[c * NB:(c + 1) * NB].reshape(128, COLS)
        m = {}
        for h in range(NCH):
            sl = slice(BOUNDS[h], BOUNDS[h + 1])
            m[f"in{h}"] = np.ascontiguousarray(
                np.concatenate([lv_t[:, sl], mu_t[:, sl], x_t[:, sl]], axis=1)
            )
        in_maps.append(m)
    return in_maps


def combine(results) -> np.ndarray:
    nq = len(QUANT)
    tot = np.zeros((nq, 128), dtype=np.float64)
    for r in results:
        a = np.asarray(r["accs"], dtype=np.float64)  # (128, nq*NCH)
        for q in range(nq):
            tot[q] += a[:, q * NCH:(q + 1) * NCH].sum(axis=1)
    P = tot[QUANT.index("P")].sum()
    vec = {q: tot[QUANT.index(q)].reshape(NB, D).sum(axis=0)
           for q in ("Sx", "Sm", "Sx2", "Sinvv")}
    # Sm is -2*S_muinvv, so  S_invv.S_x2 - 2*S_muinvv.S_x  =  Si.Sx2 + Sm.Sx
    loss = (-0.5 / N * P
            + 0.5 / float(N) ** 2 * (vec["Sinvv"] @ vec["Sx2"]
                                     + vec["Sm"] @ vec["Sx"]))
    return np.array(loss, dtype=np.float32)


def kernel(x, mu, logvar, **_kwargs):
    nc = _get_nc()
    in_maps = make_in_maps(x, mu, logvar)
    res = run_bass_kernel_spmd(nc, in_maps, list(range(N_CORES)))
    return combine(res.results)
